# revision 12
# baseline (speedup 1.0000x reference)
"""Trainium2 Bass kernel v3: block-recursion (O(N*M)) formulation of the
exponential-kernel multivariate Hawkes log-likelihood (B=4, N=2048, D=32).

Math
----
Events sorted by time, blocks of M=128 (NB=16 per sequence). For block q with
reference time tau_q = t[q*128]:
  lambda_i = mu[d_i] + within_i + cross_i                    (i in block q)
  within_i = sum_{j<i, j in q} ab[d_i,d_j] e^{-b[d_i,d_j](t_i-t_j)}
  cross_i  = sum_k ab[d_i,k] e^{-b[d_i,k](t_i-tau_q)} S_q[d_i,k]
  S_q[r,k] = sum_{j<q*128, d_j=k} e^{-b[r,k](tau_q-t_j)}     (D x D state)
with the recursion
  S_q = e^{-b (tau_q - tau_{q-1})} * S_{q-1} + P_{q-1},
  P_q[r,k] = sum_{j in q, d_j=k} e^{-b[r,k](tau_{q+1}-t_j)}.
This cuts exp/matmul work ~8x vs the dense N^2/2 pairwise sweep. The whole
recursion runs in ONE DVE tensor_tensor_scan over a (k-major, q-minor)
layout; the P matmuls write that layout directly via stepped output APs.

Sharding: one SPMD program on 8 cores = 4 batches x 2. Both cores of a batch
run the full recursion and the cross terms for all 16 blocks (cheap); the
within-block work (the expensive half) is split by alternating blocks via
host-arranged winT/winstr data; the compensator splits by event halves. The
device ships per-event cross sums C, within sums lamW, and compensator sums;
the host (during the gather) adds mu[d_i], takes logs, and folds the
alpha/mu*T constants — 13KB per core, negligible.

All exponent matmuls are fp16; block-local time offsets keep |b*t| small
(measured ~3e-5 end-to-end error vs the 2e-2 tolerance).
"""

import numpy as np
from contextlib import ExitStack

import concourse.bass as bass
import concourse.bacc as bacc
import concourse.mybir as mybir
import concourse.tile as tile
from concourse.bass_utils import run_bass_kernel_spmd

F32 = mybir.dt.float32
F16 = mybir.dt.float16
BF16 = mybir.dt.bfloat16
AF = mybir.ActivationFunctionType
F16NP = np.float16

B, N, D = 4, 2048, 32
M = 128
NB = N // M          # 16 blocks
NQ = NB - 1          # 15 recursion steps
OWNB = NB // 2       # 8 within-blocks per core

MASK_NEG = -30000.0
NOUT = NB + OWNB + 1  # C cols + lamW cols + negexp col

_PROGRAM = None


def _build_program():
    nc = bacc.Bacc("TRN2", target_bir_lowering=False, debug=False, num_devices=8)

    def din(name, shape, dt=F16):
        return nc.dram_tensor(name, shape, dt, kind="ExternalInput").ap()

    wstr = din("wstr", [D, NQ * M])          # onehot*(t - tau_{q+1}), q=0..14
    ctab = din("ctab", [64, 64])             # [lnab;-b] cols 0:32, bT32 32:64
    ohT = din("ohT", [M, NQ * D])            # block-local onehot rows
    decay = din("decay", [D, NB * D], F32)   # scan data0, (k-major, q-minor)
    crstr = din("crstr", [64, N])            # [oh; oh*(t-tau_q)] all blocks
    winT = din("winT", [64, OWNB * M])       # [b_rows; l23] own blocks
    winstr = din("winstr", [64, OWNB * M])   # [oh*(t-tau_q); oh] own blocks
    compF = din("compF", [64, 1056])         # compensator stream + tables
    out = nc.dram_tensor("out", [M, NOUT], F32, kind="ExternalOutput").ap()

    with tile.TileContext(nc) as tc:
        with ExitStack() as ctx:
            _emit(ctx, tc, nc, wstr, ctab, ohT, decay, crstr, winT, winstr,
                  compF, out)
    nc.compile()
    return nc


def _emit(ctx, tc, nc, wstr_d, ctab_d, ohT_d, decay_d, crstr_d, winT_d,
          winstr_d, compF_d, out):
    const = ctx.enter_context(tc.tile_pool(name="const", bufs=1))
    small = ctx.enter_context(tc.tile_pool(name="small", bufs=2))
    psA = ctx.enter_context(tc.tile_pool(name="psA", bufs=1, space="PSUM"))

    # ---- t=0: PE warmup dummies + Exp table preload ----------------------
    dummy = const.tile([128, 640], BF16, tag="dummy")
    nc.vector.memset(dummy[:], 0.0)
    zwall = psA.tile([128, OWNB * M], F32, tag="zw")   # 2 banks; dummies reuse
    for r in range(2):
        nc.tensor.matmul(zwall[:, r * 512 : (r + 1) * 512], dummy[:, 0:128],
                         dummy[:, 128:640], start=True, stop=True)
    d0 = small.tile([D, 1], F32, tag="d0")
    nc.vector.memset(d0[:], 1.0)
    dact = small.tile([D, 1], F32, tag="dact")
    nc.scalar.activation(dact[:], d0[:], AF.Exp)

    # ---- t=0: repeating strict-lower mask [128, 8*128] (gpsimd) ----------
    zeros = const.tile([128, OWNB * M], F32, tag="zeros")
    nc.vector.memset(zeros[:], 0.0)
    mask_w = const.tile([128, OWNB * M], F32, tag="mask_w")
    nc.gpsimd.affine_select(
        mask_w[:].rearrange("p (b j) -> p b j", j=M),
        zeros[:].rearrange("p (b j) -> p b j", j=M),
        pattern=[[0, OWNB], [-1, M]],
        compare_op=mybir.AluOpType.is_ge, fill=MASK_NEG,
        base=-1, channel_multiplier=1)

    # ---- DMAs: wstr first on sync (it gates zT); ctab tiny on gpsimd -----
    wstr = const.tile([D, NQ * M], F16, tag="wstr")
    nc.sync.dma_start(wstr[:], wstr_d)
    ctab = const.tile([64, 64], F16, tag="ctab")
    nc.gpsimd.dma_start(ctab[:], ctab_d)
    ohT = const.tile([M, NQ * D], F16, tag="ohT")
    nc.sync.dma_start(ohT[:], ohT_d)
    decay = const.tile([D, NB * D], F32, tag="decay")
    nc.sync.dma_start(decay[:], decay_d)
    compF = const.tile([64, 1056], F16, tag="compF")
    nc.gpsimd.dma_start(compF[:], compF_d)
    crstr = const.tile([64, N], F16, tag="crstr")
    nc.gpsimd.dma_start(crstr[:], crstr_d)
    winT = const.tile([64, OWNB * M], F16, tag="winT")
    nc.gpsimd.dma_start(winT[:], winT_d)
    winstr = const.tile([64, OWNB * M], F16, tag="winstr")
    nc.gpsimd.dma_start(winstr[:], winstr_d)

    # ---- W exponents: zT_q [128j, 32r] for q = 0..14 ---------------------
    zT = psA.tile([128, NQ * D], F32, tag="zT")        # 1920B/part, 1 bank
    for q in range(NQ):
        nc.tensor.matmul(zT[:, q * D : (q + 1) * D],
                         wstr[:, q * M : (q + 1) * M],
                         ctab[0:D, 32:64], start=True, stop=True)
    WT = const.tile([128, NQ * D], F16, tag="WT")
    nc.scalar.activation(WT[:], zT[:], AF.Exp)

    # ---- compensator (early: scalar-idle window) -------------------------
    z2 = psA.tile([D, 1024], F32, tag="z2")            # 2 banks
    for r in range(2):
        sl = slice(r * 512, r * 512 + 512)
        nc.tensor.matmul(z2[:, sl], compF[:, 1024:1056], compF[:, sl],
                         start=True, stop=True)
    negexp = small.tile([D, 1], F32, tag="negexp")
    e2n = const.tile([D, 1024], F16, tag="e2n")
    nc.scalar.activation(e2n[:], z2[:], AF.Exp, accum_out=negexp[:])

    # ---- P matmuls (stepped out AP -> scan layout) + S recursion ---------
    Pb = psA.tile([D, NB * D], F32, tag="Pb")          # 2KB/part, 1 bank
    nc.vector.memset(Pb[:], 0.0)
    for q in range(NQ):
        nc.tensor.matmul(Pb[:, q + 1 :: NB], WT[:, q * D : (q + 1) * D],
                         ohT[:, q * D : (q + 1) * D], start=True, stop=True)
    S16 = const.tile([D, NB * D], F16, tag="S16")
    nc.vector.tensor_tensor_scan(S16[:], decay[:], Pb[:], 0.0,
                                 op0=mybir.AluOpType.mult,
                                 op1=mybir.AluOpType.add)

    # ---- cross exponents: zc_q [128i, 32k], all 16 blocks ----------------
    zc = psA.tile([128, NB * D], F32, tag="zc")        # 2KB/part, 1 bank
    for q in range(NB):
        nc.tensor.matmul(zc[:, q * D : (q + 1) * D],
                         crstr[:, q * M : (q + 1) * M],
                         ctab[:, 0:32], start=True, stop=True)
    E = const.tile([128, NB * D], F16, tag="E")
    nc.scalar.activation(E[:], zc[:], AF.Exp)

    # ---- within-block exponents: zw_q [128i, 128j], own blocks -----------
    for qi in range(OWNB):
        nc.tensor.matmul(zwall[:, qi * M : (qi + 1) * M],
                         winT[:, qi * M : (qi + 1) * M],
                         winstr[:, qi * M : (qi + 1) * M],
                         start=True, stop=True)
    nc.vector.tensor_add(zwall[:], zwall[:], mask_w[:])
    eW = const.tile([128, OWNB * M], F16, tag="eW")
    nc.scalar.activation(eW[:], zwall[:], AF.Exp)

    lamO = const.tile([128, NOUT], F32, tag="lamO")
    nc.vector.tensor_reduce(lamO[:, NB : NB + OWNB],
                            eW[:].rearrange("p (b j) -> p b j", j=M),
                            axis=mybir.AxisListType.X, op=mybir.AluOpType.add)

    # ---- gather S per block: G_q[i,k] = S_q[d_i,k], all 16 blocks --------
    G = psA.tile([128, NB * D], F32, tag="G")          # 2KB/part, 1 bank
    for q in range(NB):
        nc.tensor.matmul(G[:, q * D : (q + 1) * D],
                         crstr[0:D, q * M : (q + 1) * M],
                         S16[:, q::NB], start=True, stop=True)

    # ---- cross term C ----------------------------------------------------
    EG = const.tile([128, NB * D], F32, tag="EG")
    nc.vector.tensor_mul(EG[:], E[:], G[:])
    nc.vector.tensor_reduce(lamO[:, 0:NB],
                            EG[:].rearrange("p (b c) -> p b c", c=D),
                            axis=mybir.AxisListType.X, op=mybir.AluOpType.add)
    nc.vector.tensor_copy(lamO[0:D, NB + OWNB : NOUT], negexp[:])
    nc.sync.dma_start(out, lamO[:])


def _host_prep(time_points, T, mu_raw, alpha_raw, beta_raw, event_types):
    time_points = np.ascontiguousarray(np.asarray(time_points, dtype=np.float32))
    T = np.asarray(T, dtype=np.float32)
    mu_raw = np.asarray(mu_raw, dtype=np.float32).reshape(D)
    alpha_raw = np.asarray(alpha_raw, dtype=np.float32)
    beta_raw = np.asarray(beta_raw, dtype=np.float32)
    event_types = np.asarray(event_types).astype(np.int64)

    def softplus(x):
        return np.log1p(np.exp(x)).astype(np.float32)

    mu = softplus(mu_raw)
    alpha = softplus(alpha_raw)
    beta = softplus(beta_raw)
    lnab = np.log(alpha * beta).astype(np.float32)
    lnalpha = np.log(alpha).astype(np.float32)
    alpha_colsum = alpha.sum(axis=0)

    in_maps = []
    hcore = np.zeros(8, dtype=np.float64)
    mu_et = []  # per-core per-event mu for host lambda assembly
    for c in range(8):
        bb, h = c // 2, c % 2
        tp = time_points[bb]
        et = event_types[bb]
        oh = np.zeros((D, N), dtype=np.float32)
        oh[et, np.arange(N)] = 1.0
        tau = tp[::M]                               # (NB,)

        wstr = np.zeros((D, NQ * M), dtype=F16NP)
        ohT = np.zeros((M, NQ * D), dtype=F16NP)
        for q in range(NQ):
            sl = slice(q * M, (q + 1) * M)
            wstr[:, sl] = oh[:, sl] * (tp[sl] - tau[q + 1])[None, :]
            ohT[:, q * D : (q + 1) * D] = oh[:, sl].T

        ctab = np.zeros((64, 64), dtype=F16NP)
        ctab[0:D, 0:D] = lnab
        ctab[D : 2 * D, 0:D] = -beta
        ctab[0:D, 32:64] = beta.T                   # bT32[k, r] = beta[r, k]

        decay = np.zeros((D, NB * D), dtype=np.float32)
        for q in range(1, NB):
            dq = tau[q] - tau[q - 1]
            decay[:, q::NB] = np.exp(-beta * dq)

        crstr = np.zeros((64, N), dtype=F16NP)
        for q in range(NB):
            sl = slice(q * M, (q + 1) * M)
            ti = tp[sl] - tau[q]
            crstr[0:D, sl] = oh[:, sl]
            crstr[D : 2 * D, sl] = oh[:, sl] * ti[None, :]

        own = list(range(h, NB, 2))
        winT = np.zeros((64, OWNB * M), dtype=F16NP)
        winstr = np.zeros((64, OWNB * M), dtype=F16NP)
        for qi, q in enumerate(own):
            sl = slice(q * M, (q + 1) * M)
            dsl = slice(qi * M, (qi + 1) * M)
            ti = tp[sl] - tau[q]
            di = et[sl]
            b_rows = beta[di, :].T
            winT[0:D, dsl] = b_rows
            winT[D : 2 * D, dsl] = lnab[di, :].T - ti[None, :] * b_rows
            winstr[0:D, dsl] = oh[:, sl] * ti[None, :]
            winstr[D : 2 * D, dsl] = oh[:, sl]

        half = slice(h * 1024, (h + 1) * 1024)
        compF = np.zeros((64, 1056), dtype=F16NP)
        compF[0:D, 0:1024] = oh[:, half]
        compF[D : 2 * D, 0:1024] = oh[:, half] * tp[half][None, :]
        g = (lnalpha.T - T[bb] * beta.T).astype(np.float32)
        compF[0:D, 1024:1056] = g.astype(F16NP)
        compF[D : 2 * D, 1024:1056] = beta.T.astype(F16NP)

        hcore[c] = float(alpha_colsum[et[half]].sum())
        if h == 0:
            hcore[c] += float(T[bb] * mu.sum())
        mu_et.append(mu[np.asarray(et).reshape(N)])

        in_maps.append(dict(wstr=wstr, ctab=ctab, ohT=ohT, decay=decay,
                            crstr=crstr, winT=winT, winstr=winstr,
                            compF=compF))
    return in_maps, hcore, mu_et


_LAST_RESULTS = None


def kernel(time_points, T, mu_raw, alpha_raw, beta_raw, event_types,
           _trace=False):
    global _PROGRAM, _LAST_RESULTS
    if _PROGRAM is None:
        _PROGRAM = _build_program()
    nc = _PROGRAM
    in_maps, hcore, mu_et = _host_prep(time_points, T, mu_raw, alpha_raw,
                                       beta_raw, event_types)
    res = run_bass_kernel_spmd(nc, in_maps, list(range(8)), trace=_trace)
    _LAST_RESULTS = res
    partial = np.zeros(8, dtype=np.float64)
    for c in range(8):
        h = c % 2
        o = np.asarray(res.results[c]["out"], dtype=np.float64)
        C = o[:, 0:NB]                    # [128, 16] cross sums, all blocks
        lamW = o[:, NB : NB + OWNB]       # [128, 8] within sums, own blocks
        negexp_sum = o[0:D, NB + OWNB].sum()
        own = list(range(h, NB, 2))
        muv = mu_et[c].reshape(NB, M).T   # [128, 16] mu[d_i] per block col
        lam = C[:, own] + lamW + muv[:, own]
        pos = np.log(np.maximum(lam, 1e-12)).sum()
        partial[c] = pos + negexp_sum - hcore[c]
    return (partial[0::2] + partial[1::2]).astype(np.float32)
